# revision 40
# baseline (speedup 1.0000x reference)
"""Trainium2 Bass kernel for nn_AttentionFlow (trilinear attention flow layer).

Full inputs -> shard batch over 8 NeuronCores (2 batches/core) -> gather.

Per batch (C [1024,768], Q [128,768]):
  S[i,j] = w1.C_i + w2.Q_j + (C_i*w3).Q_j   (c_logit dropped from columns:
           softmax over j is invariant to per-row constants)
  C2Q = softmax_j(masked S); A = C2Q @ Q
  Q2C = softmax_i(c-masked rowmax of raw S); Bctx = Q2C @ C
  out = [C | A | C*A | C*Bctx]

Layout strategy:
  - scores per 128-row n-tile in PSUM [128, 129]; col 128 carries c_logit via
    an appended w1 column on the moving operand; q_logit and -1e9*q_mask are
    added as K=1 rank-1 matmuls (ones row (x) row-vector).
  - softmax shift = post-mask row max; exp on ACT with per-partition
    scale=(1-c_mask) (masked rows -> exp(0)=1 -> exact uniform 1/128) and
    accum_out giving row sums for free.
  - C^T (needed because the PE contracts over partitions) is produced on-chip
    by PE transposes; 1/Z is folded into the A copy / C*A fused multiply.
  - Q2C runs in column space ([128,1] per tile), goes through one PE
    transpose + gpsimd partition_all_reduce for the global softmax.
"""

from contextlib import ExitStack

import numpy as np

import concourse.bass as bass
import concourse.tile as tile
from concourse import bacc, mybir
from concourse.bass_utils import run_bass_kernel_spmd
from concourse.masks import make_identity

F32 = mybir.dt.float32
AX = mybir.AluOpType
ACTF = mybir.ActivationFunctionType

NEG = np.float32(-1e9)
NCORES = 8
NB = 2           # batches per core
N = 1024         # context length
M = 128          # query length
D = 768          # feature dim
NT = N // 128    # n-tiles per batch
KC = D // 128    # contraction chunks

_CACHE: dict = {}


def _build_program(iters: int = 1) -> bass.Bass:
    nc = bacc.Bacc("TRN2", target_bir_lowering=False, debug=False)
    C_d = nc.declare_dram_parameter("C", [NB, N, D], F32, isOutput=False)
    Q_d = nc.declare_dram_parameter("Q", [NB, M, D], F32, isOutput=False)
    cmT_d = nc.declare_dram_parameter("cmT", [NB, 128, NT], F32, isOutput=False)
    qmN_d = nc.declare_dram_parameter("qmN", [NB, 1, M + 1], F32, isOutput=False)
    w1c_d = nc.declare_dram_parameter("w1c", [128, KC], F32, isOutput=False)
    w2r_d = nc.declare_dram_parameter("w2r", [1, D], F32, isOutput=False)
    w3c_d = nc.declare_dram_parameter("w3c", [128, KC], F32, isOutput=False)
    out_d = nc.declare_dram_parameter("out", [NB, N, 4 * D], F32, isOutput=True)

    with ExitStack() as ctx:
        tc = ctx.enter_context(tile.TileContext(nc))
        consts = ctx.enter_context(tc.tile_pool(name="consts", bufs=1))
        cpool = ctx.enter_context(tc.tile_pool(name="cpool", bufs=2))
        ctpool = ctx.enter_context(tc.tile_pool(name="ctpool", bufs=2 * KC))
        qpool = ctx.enter_context(tc.tile_pool(name="qpool", bufs=2))
        epool = ctx.enter_context(tc.tile_pool(name="epool", bufs=4))
        spool = ctx.enter_context(tc.tile_pool(name="spool", bufs=4))
        stA = ctx.enter_context(tc.tile_pool(name="stA", bufs=4))
        stB = ctx.enter_context(tc.tile_pool(name="stB", bufs=3))
        ps_t = ctx.enter_context(tc.tile_pool(name="ps_t", bufs=2, space="PSUM"))
        ps_s = ctx.enter_context(tc.tile_pool(name="ps_s", bufs=2, space="PSUM"))
        ps_a = ctx.enter_context(tc.tile_pool(name="ps_a", bufs=2, space="PSUM"))

        ident = consts.tile([128, 128], F32)
        make_identity(nc, ident)
        ones_row = consts.tile([1, 128], F32)
        nc.vector.memset(ones_row, 1.0)
        ones_col = consts.tile([128, 1], F32)
        nc.vector.memset(ones_col, 1.0)
        w1cols = consts.tile([128, KC], F32)
        nc.scalar.dma_start(out=w1cols, in_=w1c_d[:, :])
        w3cols = consts.tile([128, KC], F32)
        nc.scalar.dma_start(out=w3cols, in_=w3c_d[:, :])
        w2row = consts.tile([1, D], F32)
        nc.scalar.dma_start(out=w2row, in_=w2r_d[:, :])
        # broadcast w2 row to all 128 partitions via ones (x) row matmul
        w2bc_ps = ps_a.tile([128, D], F32, tag="aps", bufs=2)
        nc.tensor.matmul(w2bc_ps[:, 0:512], lhsT=ones_row, rhs=w2row[:, 0:512],
                         start=True, stop=True)
        nc.tensor.matmul(w2bc_ps[:, 512:D], lhsT=ones_row, rhs=w2row[:, 512:D],
                         start=True, stop=True)
        w2bc = consts.tile([128, D], F32)
        nc.vector.tensor_copy(out=w2bc, in_=w2bc_ps)

        loop_ctx = tc.For_i(0, iters, 1) if iters > 1 else None
        if loop_ctx is not None:
            ctx.enter_context(loop_ctx)
        for b in range(NB):
            # ------- loads (ACT HWDGE ring; stores use SP HWDGE ring) -------
            c_big = cpool.tile([128, NT, D], F32, tag="c")
            nc.scalar.dma_start(
                out=c_big, in_=C_d[b].rearrange("(t p) d -> p t d", p=128))
            c_tiles = [c_big[:, t, :] for t in range(NT)]
            q_tile = qpool.tile([128, D], F32, tag="q")
            nc.scalar.dma_start(out=q_tile, in_=Q_d[b])
            cmT = spool.tile([128, NT], F32, tag="cmT")
            nc.scalar.dma_start(out=cmT, in_=cmT_d[b])
            qmN = spool.tile([1, M + 1], F32, tag="qmN")
            nc.scalar.dma_start(out=qmN, in_=qmN_d[b])

            # mask derivations: s0=1-cm, negs0=cm-1, cmN=-1e9*cm
            s0c = spool.tile([128, NT], F32, tag="s0c")
            nc.vector.tensor_scalar(out=s0c, in0=cmT, scalar1=-1.0, scalar2=1.0,
                                    op0=AX.mult, op1=AX.add)
            negs0c = spool.tile([128, NT], F32, tag="negs0c")
            nc.vector.tensor_scalar_add(out=negs0c, in0=cmT, scalar1=-1.0)
            cmNc = spool.tile([128, NT], F32, tag="cmNc")
            nc.vector.tensor_scalar_mul(out=cmNc, in0=cmT, scalar1=float(NEG))

            # ---------------- q side ----------------
            # q_logit column then transpose to a row
            qlcol = spool.tile([128, 1], F32, tag="qlcol")
            scr = qpool.tile([128, D], F32, tag="scr")
            nc.vector.tensor_mul(out=scr, in0=q_tile, in1=w2bc)
            nc.vector.reduce_sum(out=qlcol, in_=scr, axis=mybir.AxisListType.X)
            ql_ps = ps_t.tile([1, 128], F32, tag="pst")
            nc.tensor.transpose(ql_ps, qlcol, ident)
            qrow = qpool.tile([1, M + 1], F32, tag="qrow")
            nc.vector.memset(qrow, 0.0)
            nc.vector.tensor_copy(out=qrow[:, 0:M], in_=ql_ps)

            # Q^T * w3 (+ w1 aug column) per contraction chunk
            qw3aug = qpool.tile([128, KC, M + 1], F32, tag="qw3aug")
            for c in range(KC):
                qt_ps = ps_t.tile([128, 128], F32, tag="pst")
                nc.tensor.transpose(qt_ps, q_tile[:, c * 128:(c + 1) * 128], ident)
                nc.vector.tensor_scalar_mul(out=qw3aug[:, c, 0:M], in0=qt_ps,
                                            scalar1=w3cols[:, c:c + 1])
                nc.vector.tensor_copy(out=qw3aug[:, c, M:M + 1],
                                      in_=w1cols[:, c:c + 1])

            # ---- per n-tile: C^T transposes run LOOKAHEAD tiles ahead of the
            # score/softmax/A stage so the store stream builds a backlog ----
            ct = [ctpool.tile([128, N], F32, tag="ct", name=f"ct{b}_{c}")
                  for c in range(KC)]
            G = spool.tile([128, NT], F32, tag="G")
            zr_tiles = []
            import os
            LOOKAHEAD = int(os.environ.get('K_LOOKAHEAD', '2'))
            for step in range(NT + LOOKAHEAD):
                tt = step
                if tt < NT:
                    for c in range(KC):
                        ct_ps = ps_t.tile([128, 128], F32, tag="pst")
                        nc.tensor.transpose(
                            ct_ps, c_tiles[tt][:, c * 128:(c + 1) * 128], ident)
                        dst = ct[c][:, tt * 128:(tt + 1) * 128]
                        if (tt + c) % 3 == 2:
                            nc.scalar.copy(out=dst, in_=ct_ps)
                        else:
                            nc.vector.tensor_copy(out=dst, in_=ct_ps)
                t = step - LOOKAHEAD
                if t < 0:
                    continue
                s_ps = ps_s.tile([128, M + 1], F32, tag="sps")
                for c in range(KC):
                    nc.tensor.matmul(s_ps, lhsT=ct[c][:, t * 128:(t + 1) * 128],
                                     rhs=qw3aug[:, c, :], start=(c == 0), stop=False)
                nc.tensor.matmul(s_ps, lhsT=ones_row, rhs=qrow, start=False,
                                 stop=True)
                rawmax = spool.tile([128, 1], F32, tag="rawmax")
                nc.vector.reduce_max(out=rawmax, in_=s_ps[:, 0:M],
                                     axis=mybir.AxisListType.X)
                nc.tensor.matmul(s_ps, lhsT=ones_row, rhs=qmN, start=False,
                                 stop=True, skip_group_check=True)
                shmax = spool.tile([128, 1], F32, tag="shmax")
                nc.vector.reduce_max(out=shmax, in_=s_ps[:, 0:M],
                                     axis=mybir.AxisListType.X)
                biasT = spool.tile([128, 1], F32, tag="biasT")
                nc.vector.tensor_scalar_mul(out=biasT, in0=shmax,
                                            scalar1=negs0c[:, t:t + 1])
                E = epool.tile([128, M], F32, tag="E")
                Zrow = spool.tile([128, 1], F32, tag="Zrow")
                nc.scalar.activation(out=E, in_=s_ps[:, 0:M], func=ACTF.Exp,
                                     bias=biasT, scale=s0c[:, t:t + 1],
                                     accum_out=Zrow)
                zr = spool.tile([128, 1], F32, tag="zr")
                nc.vector.reciprocal(out=zr, in_=Zrow)
                zr_tiles.append(zr)

                # q2c column: (rawmax + c_logit)*s0 - 1e9*cm
                rawc = spool.tile([128, 1], F32, tag="rawc")
                nc.vector.tensor_add(out=rawc, in0=rawmax, in1=s_ps[:, M:M + 1])
                nc.vector.tensor_scalar(out=G[:, t:t + 1], in0=rawc,
                                        scalar1=s0c[:, t:t + 1],
                                        scalar2=cmNc[:, t:t + 1],
                                        op0=AX.mult, op1=AX.add)

                # A = (E @ Q) * zr ; C*A fused
                et_ps = ps_t.tile([128, M], F32, tag="pst")
                nc.tensor.transpose(et_ps, E, ident)
                et = epool.tile([128, M], F32, tag="et")
                nc.vector.tensor_copy(out=et, in_=et_ps)
                a_ps = ps_a.tile([128, D], F32, tag="aps")
                nc.tensor.matmul(a_ps[:, 0:512], lhsT=et, rhs=q_tile[:, 0:512],
                                 start=True, stop=True)
                nc.tensor.matmul(a_ps[:, 512:D], lhsT=et, rhs=q_tile[:, 512:D],
                                 start=True, stop=True)
                stage = stA.tile([128, 3 * D], F32, tag="stA")
                nc.gpsimd.tensor_copy(out=stage[:, 0:D], in_=c_tiles[t])
                nc.scalar.activation(out=stage[:, D:2 * D], in_=a_ps,
                                     func=ACTF.Copy, scale=zr)
                ca_eng = nc.gpsimd if t % 2 == 0 else nc.vector
                ca_eng.tensor_mul(out=stage[:, 2 * D:3 * D],
                                  in0=stage[:, D:2 * D], in1=c_tiles[t])
                nc.sync.dma_start(out=out_d[b, t * 128:(t + 1) * 128, 0:3 * D],
                                  in_=stage)

            # ---------------- Q2C global softmax + Bctx ----------------
            gt_ps = ps_t.tile([NT, 128], F32, tag="pst")
            nc.tensor.transpose(gt_ps, G, ident)
            gts = spool.tile([NT, 128], F32, tag="gts")
            nc.vector.tensor_copy(out=gts, in_=gt_ps)
            m8 = spool.tile([NT, 1], F32, tag="m8")
            nc.vector.reduce_max(out=m8, in_=gts, axis=mybir.AxisListType.X)
            # global max: transpose [8,1] -> [1,8], reduce, negate, broadcast
            m8t_ps = ps_t.tile([1, NT], F32, tag="pst")
            nc.tensor.transpose(m8t_ps, m8, ident[0:NT, 0:NT])
            Mg = spool.tile([1, 1], F32, tag="Mg")
            nc.vector.reduce_max(out=Mg, in_=m8t_ps, axis=mybir.AxisListType.X)
            negMg = spool.tile([1, 1], F32, tag="negMg")
            nc.vector.tensor_scalar_mul(out=negMg, in0=Mg, scalar1=-1.0)
            nm8_ps = ps_t.tile([NT, 1], F32, tag="pst")
            nc.tensor.matmul(nm8_ps, lhsT=ones_row[:, 0:NT], rhs=negMg,
                             start=True, stop=True)
            nm8 = spool.tile([NT, 1], F32, tag="nm8")
            nc.vector.tensor_copy(out=nm8, in_=nm8_ps)
            er8 = spool.tile([NT, 128], F32, tag="er8")
            zq8 = spool.tile([NT, 1], F32, tag="zq8")
            nc.scalar.activation(out=er8, in_=gts, func=ACTF.Exp, bias=nm8,
                                 accum_out=zq8)
            # global sum: ones (x) column matmul, then reciprocal
            zq_ps = ps_t.tile([1, 1], F32, tag="pst")
            nc.tensor.matmul(zq_ps, lhsT=zq8, rhs=ones_col[0:NT, :],
                             start=True, stop=True)
            zqr = spool.tile([1, 1], F32, tag="zqr")
            nc.vector.reciprocal(out=zqr, in_=zq_ps)
            ec_ps = ps_t.tile([128, NT], F32, tag="pst")
            nc.tensor.transpose(ec_ps, er8, ident[0:NT, 0:NT])
            ecol = spool.tile([128, NT], F32, tag="ecol")
            nc.vector.tensor_copy(out=ecol, in_=ec_ps)
            bctx_ps = ps_a.tile([1, D], F32, tag="aps")
            for t in range(NT):
                nc.tensor.matmul(bctx_ps[:, 0:512], lhsT=ecol[:, t:t + 1],
                                 rhs=c_tiles[t][:, 0:512], start=(t == 0),
                                 stop=(t == NT - 1))
                nc.tensor.matmul(bctx_ps[:, 512:D], lhsT=ecol[:, t:t + 1],
                                 rhs=c_tiles[t][:, 512:D], start=(t == 0),
                                 stop=(t == NT - 1))
            bctx = spool.tile([1, D], F32, tag="bctx")
            nc.scalar.activation(out=bctx, in_=bctx_ps, func=ACTF.Copy, scale=zqr)
            # broadcast bctx row to 128 partitions (PE), then to SBUF for POOL
            bb_ps = ps_a.tile([128, D], F32, tag="aps")
            nc.tensor.matmul(bb_ps[:, 0:512], lhsT=ones_row, rhs=bctx[:, 0:512],
                             start=True, stop=True)
            nc.tensor.matmul(bb_ps[:, 512:D], lhsT=ones_row, rhs=bctx[:, 512:D],
                             start=True, stop=True)
            Bb = qpool.tile([128, D], F32, tag="Bb")
            nc.scalar.copy(out=Bb, in_=bb_ps)
            for t in range(NT):
                sb = stB.tile([128, D], F32, tag="stB")
                cb_eng = nc.gpsimd if t % 2 == 1 else nc.vector
                cb_eng.tensor_mul(out=sb, in0=c_tiles[t], in1=Bb)
                nc.sync.dma_start(out=out_d[b, t * 128:(t + 1) * 128, 3 * D:4 * D],
                                  in_=sb)
    nc.compile()
    return nc


def _get_program() -> bass.Bass:
    if "nc" not in _CACHE:
        _CACHE["nc"] = _build_program()
    return _CACHE["nc"]


def _make_in_maps(inputs) -> list:
    C = np.ascontiguousarray(np.asarray(inputs["C"], dtype=np.float32))
    Q = np.ascontiguousarray(np.asarray(inputs["Q"], dtype=np.float32))
    c_mask = np.asarray(inputs["c_mask"])
    q_mask = np.asarray(inputs["q_mask"])
    w1 = np.asarray(inputs["w1"], dtype=np.float32).reshape(-1)
    w2 = np.asarray(inputs["w2"], dtype=np.float32).reshape(-1)
    w3 = np.asarray(inputs["w3"], dtype=np.float32).reshape(-1)
    B = C.shape[0]

    # cmT[b, p, t] = c_mask[b, 0, t*128 + p]
    cmT = np.ascontiguousarray(
        c_mask[:, 0, :].astype(np.float32).reshape(B, NT, 128).transpose(0, 2, 1))
    qmN = np.zeros((B, 1, M + 1), np.float32)
    qmN[:, 0, 0:M] = q_mask[:, 0, :].astype(np.float32) * NEG
    w1c = np.ascontiguousarray(w1.reshape(KC, 128).T)
    w3c = np.ascontiguousarray(w3.reshape(KC, 128).T)
    w2r = np.ascontiguousarray(w2.reshape(1, D))

    in_maps = []
    for core in range(NCORES):
        sl = slice(core * NB, (core + 1) * NB)
        in_maps.append({
            "C": C[sl], "Q": Q[sl],
            "cmT": np.ascontiguousarray(cmT[sl]),
            "qmN": np.ascontiguousarray(qmN[sl]),
            "w1c": w1c, "w2r": w2r, "w3c": w3c,
        })
    return in_maps


def kernel(**inputs) -> np.ndarray:
    nc = _get_program()
    in_maps = _make_in_maps(inputs)
    res = run_bass_kernel_spmd(nc, in_maps, list(range(NCORES)))
    return np.concatenate([r["out"] for r in res.results], axis=0)
